# revision 19
# baseline (speedup 1.0000x reference)
"""DiffFDN Trainium2 kernel, v4: all-SBUF fp16 history + gpsimd local_scatter.

Per core (4 items, lockstep): the 48000-step FDN scan runs as 94 blocks of
L=504 timesteps. History lives entirely in SBUF as a doubled ring of 16
slots ([128, 8064] fp16, rows 0-63 nxt series, 64-67 the y series; slot s
holds block s mod 8, duplicated at slot+8 so any <=5-slot window is a
contiguous column slice). Realignment of the 16 per-line delay taps uses
InstLocalScatter (per-partition int16 index tables, Pool engine): per block
pair {b, b+1} one BIG scatter assembles everything sourced from stages
<= b-2 into SB [128,1008] and one SMALL scatter adds stage b-1's
contribution for block b+1 into SS [128,504]. Both outputs are zero-filled
off their written lanes, so two accumulating matmuls (lhsT^T @ SB-half +
lhsT^T @ SS) reconstruct the exact tap matrix by linearity. No DMA on the
recurrence chain; y drains from ring rows 64-67 via periodic HWDGE DMAs.
"""

import numpy as np

SR = 48000
IR_LEN = 48000
DELAYS = [1009, 1123, 1231, 1321, 1433, 1543, 1657, 1777, 1879, 1987,
          2081, 2179, 2287, 2383, 2503, 2617]
N = 16
FEAT = 256
BATCH = 32
NCORES = 8
IPC = BATCH // NCORES          # items per core
L = 504
NBLK = 96                      # blocks 0,1 are identically zero (d_min=1009)
RING = 8                       # ring slots (doubled to 16 in storage)
RW = RING * L                  # 4032, primary ring width
NTAP = 6                       # longest delay lines fed by direct matmul taps
TAPS = list(range(N - NTAP, N))
DMAX = DELAYS[N - NTAP - 1]    # largest delay still handled by BIG (2287)
BIGLO = DMAX + 1               # 2288: BIG window [n0-BIGLO, n0-504), even
BIGW = BIGLO - L               # 1784
BIGOFF = 5 * L - BIGLO         # 232: window start within slot b-5
SMALLW = L                     # SMALL window [n0-504, n0) = stage b-1 exactly
M_OUT = IPC * N + IPC          # 68 psum rows (64 nxt + 4 y)

_BUILT = None


def _expm64(M):
    M = M.astype(np.float64)
    nrm = np.linalg.norm(M, ord=np.inf)
    k = max(0, int(np.ceil(np.log2(max(nrm, 1e-30)))) + 2)
    Ms = M / (2.0 ** k)
    E = np.eye(M.shape[0]) + Ms
    term = Ms.copy()
    for i in range(2, 18):
        term = term @ Ms / i
        E = E + term
    for _ in range(k):
        E = E @ E
    return E


def _prologue(x, WA, bA, WB, bB, WC, bC):
    x = np.asarray(x, np.float32)
    feat = x.mean(axis=1)
    A = np.tanh(feat @ np.asarray(WA).T + bA).reshape(-1, N, N)
    Bv = np.tanh(feat @ np.asarray(WB).T + bB)
    Cv = np.tanh(feat @ np.asarray(WC).T + bC)
    S = np.triu(A, 1)
    S = S - np.swapaxes(S, -1, -2)
    g = 10.0 ** (-3.0 / SR)
    G = g ** np.asarray(DELAYS, np.float64)
    A_g = np.stack([_expm64(S[b]) for b in range(S.shape[0])])
    A_g = (A_g * G[None, None, :]).astype(np.float32)
    return A_g, Bv.astype(np.float32), Cv.astype(np.float32)


def _core_inputs(A_g4, Bv4, Cv4):
    """lhsT [64, 68] fp16: contraction row r=4i+j (line i item j) ->
    psum rows m=4i'+j (nxt) and m=64+j (y)."""
    lhsT = np.zeros((IPC * N, M_OUT), np.float32)
    bv = np.zeros((IPC * N, 1), np.float32)
    for j in range(IPC):
        for i in range(N):
            r = 4 * i + j
            for ip in range(N):
                lhsT[r, 4 * ip + j] = A_g4[j, ip, i]
            lhsT[r, IPC * N + j] = Cv4[j, i]
            bv[r, 0] = Bv4[j, i]
    # per-tap weights: full 64-row operand with only that line's rows live
    # (matmul base partition must be 0/32/64, so taps contract all 64 rows)
    lhsTt = np.zeros((IPC * N, NTAP * M_OUT), np.float32)
    for t_i, i in enumerate(TAPS):
        rows = slice(4 * i, 4 * i + 4)
        lhsTt[rows, t_i * M_OUT:(t_i + 1) * M_OUT] = lhsT[rows, :]
    return lhsT.astype(np.float16), lhsTt.astype(np.float16), bv.astype(np.float16)


def _idx_tables():
    """Per-partition scatter tables (block-phase independent).

    BIG: data col k <-> time t = n0-BIGLO+k; row r (line i_r) valid when
    dst = k - BIGLO + d_r lands in [0, 1008) (covers block b cols 0-503 and
    block b+1 cols 504-1007, all sourced from stages <= b-2; the window's
    2B base must be 4B-aligned for the Q7 uint32-pair reads). Tap lines are
    excluded (handled by direct matmuls on ring slices).
    SMALL: data col k <-> t = n0-504+k (stage b-1); dst = k + d_r - 1008
    in [0, 504) covers block b+1's tail."""
    idxb = np.full((128, BIGW), -1, np.int16)
    idxs = np.full((128, SMALLW), -1, np.int16)
    for i in range(N):
        d = DELAYS[i]
        if i in TAPS:
            continue
        for j in range(IPC):
            r = 4 * i + j
            for k in range(BIGW):
                dst = k - BIGLO + d
                if 0 <= dst < 2 * L:
                    idxb[r, k] = dst
            for k in range(SMALLW):
                dst = k + d - 1008
                if 0 <= dst < L:
                    idxs[r, k] = dst
    return idxb, idxs


def _build():
    global _BUILT
    if _BUILT is not None:
        return _BUILT
    import concourse.bacc as bacc
    import concourse.mybir as mybir
    import concourse.tile as tile

    fp32 = mybir.dt.float32
    fp16 = mybir.dt.float16
    i16 = mybir.dt.int16
    nc = bacc.Bacc("TRN2", target_bir_lowering=False, debug=False)
    lhsT_d = nc.dram_tensor("lhsT", [IPC * N, M_OUT], fp16, kind="ExternalInput")
    lhsTt_d = nc.dram_tensor("lhsTt", [IPC * N, NTAP * M_OUT], fp16, kind="ExternalInput")
    bv_d = nc.dram_tensor("bv", [IPC * N, 1], fp16, kind="ExternalInput")
    idxb_d = nc.dram_tensor("idxb", [128, BIGW], i16, kind="ExternalInput")
    idxs_d = nc.dram_tensor("idxs", [128, SMALLW], i16, kind="ExternalInput")
    y_d = nc.dram_tensor("y", [IPC, NBLK * L], fp16, kind="ExternalOutput")

    with tile.TileContext(nc) as tc:
        with tc.tile_pool(name="const", bufs=1) as cpool, \
             tc.tile_pool(name="sb", bufs=4) as sbpool, \
             tc.tile_pool(name="ss", bufs=4) as sspool, \
             tc.tile_pool(name="ps", bufs=6, space="PSUM") as ppool:
            lhsT = cpool.tile([IPC * N, M_OUT], fp16)
            nc.sync.dma_start(lhsT[:, :], lhsT_d[:, :])
            lhsTt = cpool.tile([IPC * N, NTAP * M_OUT], fp16)
            nc.sync.dma_start(lhsTt[:, :], lhsTt_d[:, :])
            idxb = cpool.tile([128, BIGW], i16)
            nc.scalar.dma_start(idxb[:, :], idxb_d[:, :])
            idxs = cpool.tile([128, SMALLW], i16)
            nc.sync.dma_start(idxs[:, :], idxs_d[:, :])

            ring = cpool.tile([128, 2 * RW], fp16)
            # first BIG (b=2) reads [2958, 4536): zero that region first so
            # the pipeline starts ~2.5us earlier
            nc.vector.memset(ring[:, 5 * L:RW], 0.0)
            nc.gpsimd.memset(ring[:, RW:RW + L], 0.0)
            nc.vector.memset(ring[:, 0:5 * L], 0.0)
            nc.gpsimd.memset(ring[:, RW + L:2 * RW], 0.0)
            # impulse: nxt(0) = Bv at t=0 (slot 0 col 0, both images)
            nc.sync.dma_start(ring[0:IPC * N, 0:1], bv_d[:, :])
            nc.sync.dma_start(ring[0:IPC * N, RW:RW + 1], bv_d[:, :])
            # y for blocks 0,1 is identically zero
            nc.sync.dma_start(y_d[:, 0:2 * L], ring[IPC * N:M_OUT, 0:2 * L])

            for b in range(2, NBLK):
                n0 = b * L
                ps = ppool.tile([M_OUT, L], fp32)
                # direct taps first: they read old ring slices, so they
                # run under the scatter and stay off the critical chain
                for t_i, i in enumerate(TAPS):
                    d = DELAYS[i]
                    lo = n0 - d
                    sl = (lo // L) % RING
                    off = lo - (lo // L) * L
                    tb = sl * L + off
                    nc.tensor.matmul(
                        ps[:, :], lhsTt[:, t_i * M_OUT:(t_i + 1) * M_OUT],
                        ring[0:IPC * N, tb:tb + L],
                        start=(t_i == 0), stop=False)
                if b % 2 == 0:
                    # BIG scatter for pair {b, b+1}: window starts at
                    # slot (b-5) col BIGOFF  <->  t = n0 - BIGLO (4B-aligned)
                    base = ((b - 5) % RING) * L + BIGOFF
                    SB = sbpool.tile([128, 2 * L], fp16)
                    nc.gpsimd.local_scatter(
                        SB[:, :], ring[:, base:base + BIGW], idxb[:, :],
                        channels=128, num_elems=2 * L, num_idxs=BIGW)
                    nc.tensor.matmul(ps[:, :], lhsT[:, :], SB[0:IPC * N, 0:L],
                                     start=False, stop=True)
                else:
                    # SMALL scatter: data = ring slot (b-2) = stage b-1
                    # exactly (<-> t in [n0'-504, n0') for the pair's n0')
                    base = ((b - 2) % RING) * L
                    SS = sspool.tile([128, L], fp16)
                    nc.gpsimd.local_scatter(
                        SS[:, :], ring[:, base:base + SMALLW], idxs[:, :],
                        channels=128, num_elems=L, num_idxs=SMALLW)
                    nc.tensor.matmul(ps[:, :], lhsT[:, :], SS[0:IPC * N, :],
                                     start=False, stop=False)
                    nc.tensor.matmul(ps[:, :], lhsT[:, :], SB[0:IPC * N, L:2 * L],
                                     start=False, stop=True)
                # write block into both ring images (fp32 -> fp16 casts);
                # the image the +2 consumer reads is written split across
                # DVE and ACT in parallel to shorten the chain.
                slot = (b % RING) * L
                if b % 2 == 1 or b % RING >= 3:
                    fast, slow = slot, RW + slot
                else:
                    fast, slow = RW + slot, slot
                nc.vector.tensor_copy(ring[0:M_OUT, fast:fast + L], ps[:, :])
                nc.scalar.copy(ring[0:M_OUT, slow:slow + L], ps[:, :])
                # y drain: blocks [b-4, b-1] once their ring writes landed
                if b % 4 == 2 and b >= 6:
                    ybase = ((b - 4) % RING) * L
                    nc.sync.dma_start(
                        y_d[:, (b - 4) * L:b * L],
                        ring[IPC * N:M_OUT, ybase:ybase + 4 * L])
                # final blocks drain individually so the kernel tail only
                # waits on one short DMA after the last copy
                if b >= NBLK - 2:
                    eng = nc.sync if b % 2 == 0 else nc.scalar
                    ybase = (b % RING) * L
                    eng.dma_start(
                        y_d[:, b * L:(b + 1) * L],
                        ring[IPC * N:M_OUT, ybase:ybase + L])

    nc.compile()
    _BUILT = nc
    return nc


def make_in_maps(inputs):
    A_g, Bv, Cv = _prologue(**inputs)
    idxb, idxs = _idx_tables()
    in_maps = []
    for k in range(NCORES):
        sl = slice(k * IPC, (k + 1) * IPC)
        lhsT, lhsTt, bv = _core_inputs(A_g[sl], Bv[sl], Cv[sl])
        in_maps.append({"lhsT": lhsT, "lhsTt": lhsTt, "bv": bv,
                        "idxb": idxb, "idxs": idxs})
    return in_maps


def kernel(x, WA, bA, WB, bB, WC, bC):
    from concourse import bass_utils

    in_maps = make_in_maps(
        {"x": x, "WA": WA, "bA": bA, "WB": WB, "bB": bB, "WC": WC, "bC": bC})
    nc = _build()
    res = bass_utils.run_bass_kernel_spmd(nc, in_maps, core_ids=list(range(NCORES)))
    y = np.concatenate(
        [res.results[k]["y"][:, :IR_LEN] for k in range(NCORES)], axis=0)
    return y[:, None, :].astype(np.float32)


# revision 20
# speedup vs baseline: 1.0237x; 1.0237x over previous
"""DiffFDN Trainium2 kernel, v4: all-SBUF fp16 history + gpsimd local_scatter.

Per core (4 items, lockstep): the 48000-step FDN scan runs as 94 blocks of
L=504 timesteps. History lives entirely in SBUF as a doubled ring of 16
slots ([128, 8064] fp16, rows 0-63 nxt series, 64-67 the y series; slot s
holds block s mod 8, duplicated at slot+8 so any <=5-slot window is a
contiguous column slice). Realignment of the 16 per-line delay taps uses
InstLocalScatter (per-partition int16 index tables, Pool engine): per block
pair {b, b+1} one BIG scatter assembles everything sourced from stages
<= b-2 into SB [128,1008] and one SMALL scatter adds stage b-1's
contribution for block b+1 into SS [128,504]. Both outputs are zero-filled
off their written lanes, so two accumulating matmuls (lhsT^T @ SB-half +
lhsT^T @ SS) reconstruct the exact tap matrix by linearity. No DMA on the
recurrence chain; y drains from ring rows 64-67 via periodic HWDGE DMAs.
"""

import numpy as np

SR = 48000
IR_LEN = 48000
DELAYS = [1009, 1123, 1231, 1321, 1433, 1543, 1657, 1777, 1879, 1987,
          2081, 2179, 2287, 2383, 2503, 2617]
N = 16
FEAT = 256
BATCH = 32
NCORES = 8
IPC = BATCH // NCORES          # items per core
L = 504
NBLK = 96                      # blocks 0,1 are identically zero (d_min=1009)
RING = 8                       # ring slots (doubled to 16 in storage)
RW = RING * L                  # 4032, primary ring width
NTAP = 6                       # longest delay lines fed by direct matmul taps
TAPS = list(range(N - NTAP, N))
DMAX = DELAYS[N - NTAP - 1]    # largest delay still handled by BIG (2287)
BIGLO = DMAX + 1               # 2288: BIG window [n0-BIGLO, n0-504), even
BIGW = BIGLO - L               # 1784
BIGOFF = 5 * L - BIGLO         # 232: window start within slot b-5
SMALLW = L                     # SMALL window [n0-504, n0) = stage b-1 exactly
M_OUT = IPC * N + IPC          # 68 psum rows (64 nxt + 4 y)

_BUILT = None


def _expm64(M):
    M = M.astype(np.float64)
    nrm = np.linalg.norm(M, ord=np.inf)
    k = max(0, int(np.ceil(np.log2(max(nrm, 1e-30)))) + 2)
    Ms = M / (2.0 ** k)
    E = np.eye(M.shape[0]) + Ms
    term = Ms.copy()
    for i in range(2, 18):
        term = term @ Ms / i
        E = E + term
    for _ in range(k):
        E = E @ E
    return E


def _prologue(x, WA, bA, WB, bB, WC, bC):
    x = np.asarray(x, np.float32)
    feat = x.mean(axis=1)
    A = np.tanh(feat @ np.asarray(WA).T + bA).reshape(-1, N, N)
    Bv = np.tanh(feat @ np.asarray(WB).T + bB)
    Cv = np.tanh(feat @ np.asarray(WC).T + bC)
    S = np.triu(A, 1)
    S = S - np.swapaxes(S, -1, -2)
    g = 10.0 ** (-3.0 / SR)
    G = g ** np.asarray(DELAYS, np.float64)
    A_g = np.stack([_expm64(S[b]) for b in range(S.shape[0])])
    A_g = (A_g * G[None, None, :]).astype(np.float32)
    return A_g, Bv.astype(np.float32), Cv.astype(np.float32)


def _core_inputs(A_g4, Bv4, Cv4):
    """lhsT [64, 68] fp16: contraction row r=4i+j (line i item j) ->
    psum rows m=4i'+j (nxt) and m=64+j (y)."""
    lhsT = np.zeros((IPC * N, M_OUT), np.float32)
    bv = np.zeros((IPC * N, 1), np.float32)
    for j in range(IPC):
        for i in range(N):
            r = 4 * i + j
            for ip in range(N):
                lhsT[r, 4 * ip + j] = A_g4[j, ip, i]
            lhsT[r, IPC * N + j] = Cv4[j, i]
            bv[r, 0] = Bv4[j, i]
    # per-tap weights: full 64-row operand with only that line's rows live
    # (matmul base partition must be 0/32/64, so taps contract all 64 rows)
    lhsTt = np.zeros((IPC * N, NTAP * M_OUT), np.float32)
    for t_i, i in enumerate(TAPS):
        rows = slice(4 * i, 4 * i + 4)
        lhsTt[rows, t_i * M_OUT:(t_i + 1) * M_OUT] = lhsT[rows, :]
    return lhsT.astype(np.float16), lhsTt.astype(np.float16), bv.astype(np.float16)


def _idx_tables():
    """Per-partition scatter tables (block-phase independent).

    BIG: data col k <-> time t = n0-BIGLO+k; row r (line i_r) valid when
    dst = k - BIGLO + d_r lands in [0, 1008) (covers block b cols 0-503 and
    block b+1 cols 504-1007, all sourced from stages <= b-2; the window's
    2B base must be 4B-aligned for the Q7 uint32-pair reads). Tap lines are
    excluded (handled by direct matmuls on ring slices).
    SMALL: data col k <-> t = n0-504+k (stage b-1); dst = k + d_r - 1008
    in [0, 504) covers block b+1's tail."""
    idxb = np.full((128, BIGW), -1, np.int16)
    idxs = np.full((128, SMALLW), -1, np.int16)
    for i in range(N):
        d = DELAYS[i]
        if i in TAPS:
            continue
        for j in range(IPC):
            r = 4 * i + j
            for k in range(BIGW):
                dst = k - BIGLO + d
                if 0 <= dst < 2 * L:
                    idxb[r, k] = dst
            for k in range(SMALLW):
                dst = k + d - 1008
                if 0 <= dst < L:
                    idxs[r, k] = dst
    return idxb, idxs


def _build():
    global _BUILT
    if _BUILT is not None:
        return _BUILT
    import concourse.bacc as bacc
    import concourse.mybir as mybir
    import concourse.tile as tile

    fp32 = mybir.dt.float32
    fp16 = mybir.dt.float16
    i16 = mybir.dt.int16
    nc = bacc.Bacc("TRN2", target_bir_lowering=False, debug=False)
    lhsT_d = nc.dram_tensor("lhsT", [IPC * N, M_OUT], fp16, kind="ExternalInput")
    lhsTt_d = nc.dram_tensor("lhsTt", [IPC * N, NTAP * M_OUT], fp16, kind="ExternalInput")
    bv_d = nc.dram_tensor("bv", [IPC * N, 1], fp16, kind="ExternalInput")
    idxb_d = nc.dram_tensor("idxb", [128, BIGW], i16, kind="ExternalInput")
    idxs_d = nc.dram_tensor("idxs", [128, SMALLW], i16, kind="ExternalInput")
    y_d = nc.dram_tensor("y", [IPC, NBLK * L], fp16, kind="ExternalOutput")

    with tile.TileContext(nc) as tc:
        with tc.tile_pool(name="const", bufs=1) as cpool, \
             tc.tile_pool(name="sb", bufs=4) as sbpool, \
             tc.tile_pool(name="ss", bufs=4) as sspool, \
             tc.tile_pool(name="ps", bufs=6, space="PSUM") as ppool:
            lhsT = cpool.tile([IPC * N, M_OUT], fp16)
            nc.sync.dma_start(lhsT[:, :], lhsT_d[:, :])
            lhsTt = cpool.tile([IPC * N, NTAP * M_OUT], fp16)
            nc.sync.dma_start(lhsTt[:, :], lhsTt_d[:, :])
            idxb = cpool.tile([128, BIGW], i16)
            nc.scalar.dma_start(idxb[:, :], idxb_d[:, :])
            idxs = cpool.tile([128, SMALLW], i16)
            nc.sync.dma_start(idxs[:, :], idxs_d[:, :])

            ring = cpool.tile([128, 2 * RW], fp16)
            nc.vector.memset(ring[:, 0:RW], 0.0)
            nc.gpsimd.memset(ring[:, RW:2 * RW], 0.0)
            # impulse: nxt(0) = Bv at t=0 (slot 0 col 0, both images)
            nc.sync.dma_start(ring[0:IPC * N, 0:1], bv_d[:, :])
            nc.sync.dma_start(ring[0:IPC * N, RW:RW + 1], bv_d[:, :])
            # y for blocks 0,1 is identically zero
            nc.sync.dma_start(y_d[:, 0:2 * L], ring[IPC * N:M_OUT, 0:2 * L])

            for b in range(2, NBLK):
                n0 = b * L
                ps = ppool.tile([M_OUT, L], fp32)
                # direct taps first: they read old ring slices, so they
                # run under the scatter and stay off the critical chain
                for t_i, i in enumerate(TAPS):
                    d = DELAYS[i]
                    lo = n0 - d
                    sl = (lo // L) % RING
                    off = lo - (lo // L) * L
                    tb = sl * L + off
                    nc.tensor.matmul(
                        ps[:, :], lhsTt[:, t_i * M_OUT:(t_i + 1) * M_OUT],
                        ring[0:IPC * N, tb:tb + L],
                        start=(t_i == 0), stop=False)
                if b % 2 == 0:
                    # BIG scatter for pair {b, b+1}: window starts at
                    # slot (b-5) col BIGOFF  <->  t = n0 - BIGLO (4B-aligned)
                    base = ((b - 5) % RING) * L + BIGOFF
                    SB = sbpool.tile([128, 2 * L], fp16)
                    nc.gpsimd.local_scatter(
                        SB[:, :], ring[:, base:base + BIGW], idxb[:, :],
                        channels=128, num_elems=2 * L, num_idxs=BIGW)
                    nc.tensor.matmul(ps[:, :], lhsT[:, :], SB[0:IPC * N, 0:L],
                                     start=False, stop=True)
                else:
                    # SMALL scatter: data = ring slot (b-2) = stage b-1
                    # exactly (<-> t in [n0'-504, n0') for the pair's n0')
                    base = ((b - 2) % RING) * L
                    SS = sspool.tile([128, L], fp16)
                    nc.gpsimd.local_scatter(
                        SS[:, :], ring[:, base:base + SMALLW], idxs[:, :],
                        channels=128, num_elems=L, num_idxs=SMALLW)
                    nc.tensor.matmul(ps[:, :], lhsT[:, :], SS[0:IPC * N, :],
                                     start=False, stop=False)
                    nc.tensor.matmul(ps[:, :], lhsT[:, :], SB[0:IPC * N, L:2 * L],
                                     start=False, stop=True)
                # write block into both ring images (fp32 -> fp16 casts);
                # the image the +2 consumer reads is written split across
                # DVE and ACT in parallel to shorten the chain.
                slot = (b % RING) * L
                if b % 2 == 1 or b % RING >= 3:
                    fast, slow = slot, RW + slot
                else:
                    fast, slow = RW + slot, slot
                nc.vector.tensor_copy(ring[0:M_OUT, fast:fast + L], ps[:, :])
                nc.scalar.copy(ring[0:M_OUT, slow:slow + L], ps[:, :])
                # y drain: blocks [b-4, b-1] once their ring writes landed
                if b % 4 == 2 and b >= 6:
                    ybase = ((b - 4) % RING) * L
                    nc.sync.dma_start(
                        y_d[:, (b - 4) * L:b * L],
                        ring[IPC * N:M_OUT, ybase:ybase + 4 * L])
            # tail: blocks 94, 95 (ring slots 6, 7)
            nc.sync.dma_start(
                y_d[:, (NBLK - 2) * L:NBLK * L],
                ring[IPC * N:M_OUT, 6 * L:8 * L])
    nc.compile()
    _BUILT = nc
    return nc


def make_in_maps(inputs):
    A_g, Bv, Cv = _prologue(**inputs)
    idxb, idxs = _idx_tables()
    in_maps = []
    for k in range(NCORES):
        sl = slice(k * IPC, (k + 1) * IPC)
        lhsT, lhsTt, bv = _core_inputs(A_g[sl], Bv[sl], Cv[sl])
        in_maps.append({"lhsT": lhsT, "lhsTt": lhsTt, "bv": bv,
                        "idxb": idxb, "idxs": idxs})
    return in_maps


def kernel(x, WA, bA, WB, bB, WC, bC):
    from concourse import bass_utils

    in_maps = make_in_maps(
        {"x": x, "WA": WA, "bA": bA, "WB": WB, "bB": bB, "WC": WC, "bC": bC})
    nc = _build()
    res = bass_utils.run_bass_kernel_spmd(nc, in_maps, core_ids=list(range(NCORES)))
    y = np.concatenate(
        [res.results[k]["y"][:, :IR_LEN] for k in range(NCORES)], axis=0)
    return y[:, None, :].astype(np.float32)
